# revision 1
# baseline (speedup 1.0000x reference)
"""Trainium2 Bass kernel for nn_CaptioningRNN (attention LSTM over T=64).

Data-parallel over the batch: N=256 samples split across 8 NeuronCores
(32 samples/core), weights replicated, no collectives.

Per-core algorithm (all matmuls bf16 on the TensorEngine, state in f32):
  1. xproj phase: xpT = (x @ Wx + b) computed transposed via Wx-stationary
     matmuls, stored to a DRAM scratch laid out so the per-step slice loads
     as a clean [128, 512] tile.
  2. P phase: P[n, k, :] = A[n, :, k] @ Wattn precomputed once (the
     attention context contribution to the gates becomes a w-weighted sum
     of P rows, replacing a per-step [32,1024]@[1024,4096] matmul).
     h0 = c0 = mean_k(A) computed on device from an f32 copy of A.
  3. Recurrence (64 steps):
     - scores via hT-chunk matmuls against a permuted A (cross-sample
       products in PSUM, diagonal extracted with a mask+reduce on DVE)
     - softmax on [32,16] (ACT exp with fused row-sum)
     - w transposed (DVE 32x32 stream transpose) and expanded to the
       (k, n_g)-partition block-diagonal layout via a one-hot matmul + mask
     - gates = h @ Wh + sum_k w_k P_k accumulated into two PSUM strips
       using 4-way tensor-engine column tiling (beats the M=32 small-batch
       penalty ~4x)
     - strips transposed on PE; cell math done in h-on-partition space so
       i/f/o/g land on identical lanes (no cross-partition ops needed)
  4. Output written transposed [t, h, n]; host reassembles to (N, T, H).
"""

import numpy as np
import ml_dtypes

import concourse.bacc as bacc
import concourse.mybir as mybir
from concourse import bass_utils
from concourse.tile import TileContext

F32, BF16 = mybir.dt.float32, mybir.dt.bfloat16
AF = mybir.ActivationFunctionType
ALU = mybir.AluOpType
AX = mybir.AxisListType
BF = ml_dtypes.bfloat16

N, T, D, H = 256, 64, 1024, 1024
NCORES = 8
NL = N // NCORES          # 32 samples per core
HC = 8                    # 128-row chunks of D/H
G, GS = 4, 8              # sample groups of 8 (for the (k, n_g) 128-partition layout)
H4 = 4 * H                # 4096 gate columns

_built = None


def _consts():
    # E16[k', 8k + n] = (k' == k): one-hot expansion of wT rows onto the
    # (k-major, n_g-minor) 128-partition layout.
    e16 = np.zeros((16, 128), dtype=BF)
    for k in range(16):
        e16[k, 8 * k : 8 * k + 8] = 1
    # M32R[p, 128 g + 32 rep + m] = (m % 8 == p % 8) & (m // 8 == g):
    # block-diagonal mask producing masked_g = w[m, k(p)] only for group-g
    # samples, replicated 4x for the column-tiled matmuls.
    p = np.arange(128)[:, None]
    m = np.arange(32)[None, :]
    m32r = np.zeros((128, 512), dtype=BF)
    for g in range(4):
        blk = ((m % 8 == p % 8) & (m // 8 == g)).astype(BF)
        for rep in range(4):
            m32r[:, 128 * g + 32 * rep : 128 * g + 32 * rep + 32] = blk
    # Mdiag[n, 32 k + n'] = (n == n') / 32: extracts the diagonal of the
    # cross-sample score products and applies the 1/sqrt(H) softmax scale.
    md = np.zeros((32, 512), dtype=np.float32)
    n_ = np.arange(32)
    for k in range(16):
        md[n_, 32 * k + n_] = 1.0 / 32.0
    return e16, m32r, md


def _build_nc(t_steps=T):
    nc = bacc.Bacc(trn_type="TRN2", target_bir_lowering=False, debug=False)

    ap_xT = nc.dram_tensor("xT", [D, T * NL], BF16, kind="ExternalInput").ap()
    ap_Asc = nc.dram_tensor("Asc", [H, 512], BF16, kind="ExternalInput").ap()
    ap_Asc32 = nc.dram_tensor("Asc32", [H, 512], F32, kind="ExternalInput").ap()
    ap_Wx = nc.dram_tensor("Wx", [D, H4], BF16, kind="ExternalInput").ap()
    ap_Wh = nc.dram_tensor("Wh", [H, H4], BF16, kind="ExternalInput").ap()
    ap_Wattn = nc.dram_tensor("Wattn", [H, H4], BF16, kind="ExternalInput").ap()
    ap_bT = nc.dram_tensor("bT", [128, 32], F32, kind="ExternalInput").ap()
    outT = nc.dram_tensor("outT", [T, H, NL], F32, kind="ExternalOutput").ap()
    # xps[t, r, p, q, j, n] = xproj[t][n, j*1024 + r*512 + q*128 + p]
    xps = nc.dram_tensor("xps", [T, 2, 128, 4, 4, NL], F32, kind="Internal").ap()

    e16_np, m32r_np, md_np = _consts()
    eye_d = nc.inline_tensor(np.eye(128, dtype=np.float32), "c_eye")
    e16_d = nc.inline_tensor(e16_np, "c_e16")
    m32r_d = nc.inline_tensor(m32r_np, "c_m32r")
    md_d = nc.inline_tensor(md_np, "c_mdiag")

    with TileContext(nc) as tc:
        with tc.tile_pool(name="pers", bufs=1) as pers:
            Wh_sb = pers.tile([128, HC * H4], BF16, tag="Wh")
            Asc_sb = pers.tile([128, HC * 512], BF16, tag="Asc")
            P_sb = pers.tile([128, G * H4], BF16, tag="P")
            uTh = pers.tile([128, HC * 128], BF16, tag="uTh")
            cT = pers.tile([128, 256], F32, tag="cT")
            eye = pers.tile([128, 128], F32, tag="eye")
            E16 = pers.tile([16, 128], BF16, tag="E16")
            M32R = pers.tile([128, 512], BF16, tag="M32R")
            Mdiag = pers.tile([32, 512], F32, tag="Mdiag")
            b_sb = pers.tile([128, 32], F32, tag="bT")
            wsq = pers.tile([32, 32], F32, tag="wsq")

            nc.sync.dma_start(eye[:], eye_d.ap()[:])
            nc.sync.dma_start(E16[:], e16_d.ap()[:])
            nc.sync.dma_start(M32R[:], m32r_d.ap()[:])
            nc.sync.dma_start(Mdiag[:], md_d.ap()[:])
            nc.sync.dma_start(b_sb[:], ap_bT[:])
            nc.gpsimd.memset(wsq[:], 0.0)
            for c in range(HC):
                nc.sync.dma_start(
                    Wh_sb[:, c * H4 : (c + 1) * H4], ap_Wh[128 * c : 128 * (c + 1), :]
                )
                nc.sync.dma_start(
                    Asc_sb[:, c * 512 : (c + 1) * 512],
                    ap_Asc[128 * c : 128 * (c + 1), :],
                )

            # ---------------- phase A: xproj -> DRAM scratch ----------------
            with tc.tile_pool(name="phx1", bufs=1) as phx1, \
                 tc.tile_pool(name="phx", bufs=3) as phx, \
                 tc.tile_pool(name="psX", bufs=2, space="PSUM") as psX:
                xT_sb = phx1.tile([128, HC * T * NL], BF16, tag="xTsb")
                for c in range(HC):
                    nc.sync.dma_start(
                        xT_sb[:, c * T * NL : (c + 1) * T * NL],
                        ap_xT[128 * c : 128 * (c + 1), :],
                    )
                for W in range(32):
                    j, r, q = W // 8, (W % 8) // 4, W % 4
                    Wxb = phx.tile([128, HC * 128], BF16, tag="Wxb")
                    for c in range(HC):
                        nc.sync.dma_start(
                            Wxb[:, c * 128 : (c + 1) * 128],
                            ap_Wx[128 * c : 128 * (c + 1), 128 * W : 128 * (W + 1)],
                        )
                    for t4 in range(4):
                        psx = psX.tile([128, 512], F32, tag="psx")
                        for c in range(HC):
                            nc.tensor.matmul(
                                psx[:],
                                Wxb[:, c * 128 : (c + 1) * 128],
                                xT_sb[:, c * T * NL + 512 * t4 : c * T * NL + 512 * (t4 + 1)],
                                start=(c == 0),
                                stop=(c == HC - 1),
                            )
                        sxp = phx.tile([128, 512], F32, tag="sxp")
                        nc.vector.tensor_scalar_add(sxp[:], psx[:], b_sb[:, W : W + 1])
                        nc.sync.dma_start(
                            xps[16 * t4 : 16 * (t4 + 1), r, :, q, j, :].transpose(
                                [1, 0, 2]
                            ),
                            sxp[:].rearrange("p (t n) -> p t n", t=16),
                        )

            # ------------- phase B: P precompute + h0/c0 init -------------
            with tc.tile_pool(name="php1", bufs=1) as php1, \
                 tc.tile_pool(name="php", bufs=3) as php, \
                 tc.tile_pool(name="psP", bufs=2, space="PSUM") as psP:
                A32 = php1.tile([128, HC * 512], F32, tag="A32")
                for c in range(HC):
                    nc.sync.dma_start(
                        A32[:, c * 512 : (c + 1) * 512],
                        ap_Asc32[128 * c : 128 * (c + 1), :],
                    )
                for c in range(HC):
                    h0s = php.tile([128, 32], F32, tag="h0s")
                    nc.vector.tensor_reduce(
                        h0s[:],
                        A32[:, c * 512 : (c + 1) * 512].rearrange(
                            "p (k n) -> p n k", k=16
                        ),
                        axis=AX.X,
                        op=ALU.add,
                    )
                    nc.vector.tensor_scalar_mul(
                        cT[:, 32 * c : 32 * (c + 1)], h0s[:], 1.0 / 16.0
                    )
                    for rep in range(4):
                        nc.vector.tensor_copy(
                            uTh[:, 128 * c + 32 * rep : 128 * c + 32 * (rep + 1)],
                            cT[:, 32 * c : 32 * (c + 1)],
                        )
                # contiguous staging of the group-selected A columns so the
                # matmul stationary operand has a single free dim
                Ag = php1.tile([128, G * HC * 128], BF16, tag="Ag")
                for g in range(G):
                    for c in range(HC):
                        nc.vector.tensor_copy(
                            Ag[:, (g * HC + c) * 128 : (g * HC + c) * 128 + 128],
                            Asc_sb[:, c * 512 : (c + 1) * 512].rearrange(
                                "p (k n) -> p k n", k=16
                            )[:, :, GS * g : GS * (g + 1)],
                        )
                for blk in range(8):
                    Wab = php.tile([128, HC * 512], BF16, tag="Wab")
                    for c in range(HC):
                        nc.sync.dma_start(
                            Wab[:, c * 512 : (c + 1) * 512],
                            ap_Wattn[128 * c : 128 * (c + 1), 512 * blk : 512 * (blk + 1)],
                        )
                    for g in range(G):
                        psp = psP.tile([128, 512], F32, tag="psp")
                        for c in range(HC):
                            nc.tensor.matmul(
                                psp[:],
                                Ag[:, (g * HC + c) * 128 : (g * HC + c) * 128 + 128],
                                Wab[:, c * 512 : (c + 1) * 512],
                                start=(c == 0),
                                stop=(c == HC - 1),
                            )
                        nc.vector.tensor_copy(
                            P_sb[:, g * H4 + 512 * blk : g * H4 + 512 * (blk + 1)],
                            psp[:],
                        )

            # ---------------------- phase C: recurrence ----------------------
            with tc.tile_pool(name="wrk", bufs=2) as wrk, \
                 tc.tile_pool(name="psc", bufs=2, space="PSUM") as psc_pool, \
                 tc.tile_pool(name="pwx", bufs=1, space="PSUM") as pwx_pool, \
                 tc.tile_pool(name="pstr", bufs=1, space="PSUM") as pstr_pool, \
                 tc.tile_pool(name="paT", bufs=1, space="PSUM") as paT_pool:
                q4 = lambda ap: ap.rearrange("p (q c) -> p q c", q=4)
                for t in range(t_steps):
                    # prefetched xproj slices for this step
                    xpt = [wrk.tile([128, 512], F32, tag=f"xpt{r}", name=f"xpt{r}_{t}") for r in range(2)]
                    for r in range(2):
                        nc.sync.dma_start(xpt[r][:], xps[t, r])

                    # -- scores: cross-sample products, diag extract, softmax
                    psc = psc_pool.tile([32, 512], F32, tag="psc")
                    for c in range(HC):
                        nc.tensor.matmul(
                            psc[:],
                            uTh[:, c * 128 : c * 128 + 32],
                            Asc_sb[:, c * 512 : (c + 1) * 512],
                            start=(c == 0),
                            stop=(c == HC - 1),
                        )
                    scm = wrk.tile([32, 512], F32, tag="scm")
                    nc.vector.tensor_mul(scm[:], psc[:], Mdiag[:])
                    scores = wrk.tile([32, 16], F32, tag="scores")
                    nc.vector.tensor_reduce(
                        scores[:],
                        scm[:].rearrange("p (k n) -> p k n", k=16),
                        axis=AX.X,
                        op=ALU.add,
                    )
                    nmx = wrk.tile([32, 1], F32, tag="nmx")
                    nc.vector.tensor_reduce(
                        nmx[:], scores[:], axis=AX.X, op=ALU.max, negate=True
                    )
                    ex = wrk.tile([32, 16], F32, tag="ex")
                    esum = wrk.tile([32, 1], F32, tag="esum")
                    nc.scalar.activation(
                        ex[:], scores[:], AF.Exp, bias=nmx[:], scale=1.0,
                        accum_out=esum[:],
                    )
                    rcp = wrk.tile([32, 1], F32, tag="rcp")
                    nc.vector.reciprocal(rcp[:], esum[:])
                    nc.vector.tensor_scalar_mul(wsq[:, 0:16], ex[:], rcp[:])
                    wT = wrk.tile([32, 32], F32, tag="wT")
                    nc.vector.transpose(wT[:], wsq[:])
                    wrep = wrk.tile([16, 128], BF16, tag="wrep")
                    for rep in range(4):
                        nc.vector.tensor_copy(
                            wrep[:, 32 * rep : 32 * (rep + 1)], wT[0:16, 0:32]
                        )
                    pwx = pwx_pool.tile([128, 128], F32, tag="pwx")
                    nc.tensor.matmul(pwx[:], E16[:], wrep[:], start=True, stop=True)
                    masked = wrk.tile([128, 512], BF16, tag="masked")
                    for g in range(G):
                        nc.vector.tensor_mul(
                            masked[:, g * 128 : (g + 1) * 128],
                            pwx[:],
                            M32R[:, g * 128 : (g + 1) * 128],
                        )

                    # -- gates: h @ Wh + sum_k w_k P_k into 2 column-tiled strips
                    strips = [
                        pstr_pool.tile([128, 512], F32, tag=f"strip{r}",
                                       name=f"strip{r}_{t}")
                        for r in range(2)
                    ]
                    for c in range(HC):
                        for r in range(2):
                            for j in range(4):
                                nc.tensor.matmul(
                                    strips[r][32 * j : 32 * (j + 1), :],
                                    uTh[:, c * 128 + 32 * j : c * 128 + 32 * (j + 1)],
                                    Wh_sb[:, c * H4 + j * 1024 + r * 512 : c * H4 + j * 1024 + r * 512 + 512],
                                    start=(c == 0),
                                    stop=False,
                                    skip_group_check=True,
                                    tile_position=(0, 32 * j),
                                )
                    for r in range(2):
                        for g in range(G):
                            for j in range(4):
                                nc.tensor.matmul(
                                    strips[r][32 * j : 32 * (j + 1), :],
                                    masked[:, g * 128 + 32 * j : g * 128 + 32 * (j + 1)],
                                    P_sb[:, g * H4 + j * 1024 + r * 512 : g * H4 + j * 1024 + r * 512 + 512],
                                    start=False,
                                    stop=(g == G - 1),
                                    skip_group_check=True,
                                    tile_position=(0, 32 * j),
                                )

                    # -- transpose strips, add xproj, activations, cell update
                    for r in range(2):
                        sg = wrk.tile([128, 512], F32, tag=f"sg{r}")
                        nc.vector.tensor_copy(sg[:], strips[r][:])
                        pat = paT_pool.tile([128, 512], F32, tag=f"pat{r}")
                        for q in range(4):
                            nc.tensor.matmul(
                                pat[:, 128 * q : 128 * (q + 1)],
                                sg[:, 128 * q : 128 * (q + 1)],
                                eye[:],
                                is_transpose=True,
                                start=(q == 0),
                                stop=(q == 3),
                            )
                        ssum = wrk.tile([128, 512], F32, tag=f"ssum{r}")
                        nc.vector.tensor_add(ssum[:], pat[:], xpt[r][:])
                        act = wrk.tile([128, 512], F32, tag=f"act{r}")
                        nc.scalar.activation(
                            q4(act[:])[:, :, 0:96], q4(ssum[:])[:, :, 0:96], AF.Sigmoid
                        )
                        nc.scalar.activation(
                            q4(act[:])[:, :, 96:128], q4(ssum[:])[:, :, 96:128], AF.Tanh
                        )
                        i_v = q4(act[:])[:, :, 0:32]
                        f_v = q4(act[:])[:, :, 32:64]
                        o_v = q4(act[:])[:, :, 64:96]
                        g_v = q4(act[:])[:, :, 96:128]
                        cview = cT[:, 128 * r : 128 * (r + 1)].rearrange(
                            "p (q n) -> p q n", q=4
                        )
                        ig = wrk.tile([128, 128], F32, tag=f"ig{r}")
                        nc.vector.tensor_mul(q4(ig[:]), i_v, g_v)
                        fc = wrk.tile([128, 128], F32, tag=f"fc{r}")
                        nc.vector.tensor_mul(q4(fc[:]), f_v, cview)
                        nc.vector.tensor_add(
                            cview, q4(ig[:]), q4(fc[:])
                        )
                        tch = wrk.tile([128, 128], F32, tag=f"tch{r}")
                        nc.scalar.activation(
                            tch[:], cT[:, 128 * r : 128 * (r + 1)], AF.Tanh
                        )
                        h32 = wrk.tile([128, 128], F32, tag=f"h32{r}")
                        nc.vector.tensor_mul(
                            h32[:].rearrange("p (q n) -> p q n", q=4),
                            o_v,
                            tch[:].rearrange("p (q n) -> p q n", q=4),
                        )
                        # write h into uTh (bf16, 4 replicas) for step t+1
                        uv = uTh[:].rearrange("p (c rep n) -> p c rep n", c=HC, rep=4)
                        for rep in range(4):
                            nc.vector.tensor_copy(
                                uv[:, 4 * r : 4 * (r + 1), rep, :],
                                h32[:].rearrange("p (q n) -> p q n", q=4),
                            )
                        nc.sync.dma_start(
                            outT[t, 512 * r : 512 * (r + 1), :].rearrange(
                                "(q p) n -> p q n", p=128
                            ),
                            h32[:].rearrange("p (q n) -> p q n", q=4),
                        )
    nc.compile()
    return nc


def _prep_shards(inputs):
    x = np.asarray(inputs["x"], np.float32)
    A = np.asarray(inputs["A"], np.float32)
    Wx = np.asarray(inputs["Wx"], np.float32)
    Wh = np.asarray(inputs["Wh"], np.float32)
    Wattn = np.asarray(inputs["Wattn"], np.float32)
    b = np.asarray(inputs["b"], np.float32)

    Wx_bf = np.ascontiguousarray(Wx.astype(BF))
    Wh_bf = np.ascontiguousarray(Wh.astype(BF))
    Wa_bf = np.ascontiguousarray(Wattn.astype(BF))
    bT = np.ascontiguousarray(b.reshape(32, 128).T.astype(np.float32))

    in_maps = []
    for i in range(NCORES):
        ns = slice(NL * i, NL * (i + 1))
        xT = x[ns].transpose(2, 1, 0).reshape(D, T * NL)
        Asc = A[ns].reshape(NL, H, 16).transpose(1, 2, 0).reshape(H, 512)
        in_maps.append(
            {
                "xT": np.ascontiguousarray(xT.astype(BF)),
                "Asc": np.ascontiguousarray(Asc.astype(BF)),
                "Asc32": np.ascontiguousarray(Asc.astype(np.float32)),
                "Wx": Wx_bf,
                "Wh": Wh_bf,
                "Wattn": Wa_bf,
                "bT": bT,
            }
        )
    return in_maps


def _get_nc():
    global _built
    if _built is None:
        _built = _build_nc()
    return _built


def _run(inputs, **kwargs):
    nc = _get_nc()
    in_maps = _prep_shards(inputs)
    res = bass_utils.run_bass_kernel_spmd(
        nc, in_maps, core_ids=list(range(NCORES)), **kwargs
    )
    out = np.empty((N, T, H), np.float32)
    for i in range(NCORES):
        out[NL * i : NL * (i + 1)] = res.results[i]["outT"].transpose(2, 0, 1)
    return out, res


def kernel(**inputs):
    out, _ = _run(inputs)
    return out



# revision 6
# speedup vs baseline: 2.0203x; 2.0203x over previous
"""Trainium2 Bass kernel for nn_CaptioningRNN (attention LSTM over T=64).

Data-parallel over the batch: N=256 samples split across 8 NeuronCores
(32 samples/core), weights replicated, no collectives.

Per-core algorithm (v3):
  - No xproj prepass: x @ Wx is accumulated directly into each step's gate
    strips on the TensorEngine (x-chunk stationary [128,32], Wx moving,
    4-way column tiling), emitted one step ahead so it executes during the
    previous step's vector/scalar tail. This removes the xps DRAM scratch
    round-trip entirely.
  - Strips are laid out [32*qh + n, (j, p)]: the 4 PE column-tile quadrants
    map to 128-column blocks (qh) of the hidden dim, NOT the gate index, so
    i/f/o/g for one (n, h') live on one partition and the LSTM cell math
    runs directly in strip space (no [128,512] transposes).
  - P[n,k,:] = A[n,:,k] @ Wattn precomputed once; the bias b is folded into
    P (softmax weights sum to 1), so gates = xWx + hWh + sum_k w_k P_k
    includes +b exactly.
  - scores via 4 column-tiled accumulation chains into one [128,512] PSUM,
    diag-masked and reduced on DVE, block-summed with a tiny PE matmul.
  - softmax exp computed as sigmoid(s-m)/(1-sigmoid(s-m)) so the scalar
    engine never swaps activation tables (Exp <-> Sigmoid/Tanh reload costs
    ~2.6us/step otherwise).
  - h is produced per 512-block as [s, p] f32, DMA'd straight to the output,
    and PE-transposed ([128,128]) to hT bf16 for the next step's stationary.
  - x streamed in quarters (16 steps each) to fit SBUF next to Wx+Wh+P.
"""

import numpy as np
import ml_dtypes

import concourse.bacc as bacc
import concourse.mybir as mybir
from concourse import bass_utils
from concourse.tile import TileContext

F32, BF16 = mybir.dt.float32, mybir.dt.bfloat16
AF = mybir.ActivationFunctionType
ALU = mybir.AluOpType
AX = mybir.AxisListType
BF = ml_dtypes.bfloat16

N, T, D, H = 256, 64, 1024, 1024
NCORES = 8
NL = N // NCORES          # 32 samples per core
HC = 8                    # 128-row chunks of D/H
H4 = 4 * H                # 4096 gate columns
QT = 16                   # steps per x quarter

_built = None


def _consts():
    # E16[k', 8k + ng] = (k' == k): expands wT rows onto the (k, ng) layout.
    e16 = np.zeros((16, 128), dtype=BF)
    for k in range(16):
        e16[k, 8 * k : 8 * k + 8] = 1
    # M128[p, 32g + m] = (m % 8 == p % 8) & (m // 8 == g): group-g selector.
    p = np.arange(128)[:, None]
    m = np.arange(32)[None, :]
    m128 = np.zeros((128, 128), dtype=BF)
    for g in range(4):
        m128[:, 32 * g : 32 * (g + 1)] = ((m % 8 == p % 8) & (m // 8 == g)).astype(BF)
    # Mdiag[32b + n, 32k + n'] = (n == n') / 32: diagonal extract + 1/sqrt(H)
    # scale, replicated over the 4 partition blocks of the column-tiled psc.
    md = np.zeros((128, 512), dtype=np.float32)
    n_ = np.arange(32)
    for b in range(4):
        for k in range(16):
            md[32 * b + n_, 32 * k + n_] = 1.0 / 32.0
    # S4[32b + n', n] = (n' == n): partition-block sum via PE.
    s4 = np.zeros((128, 32), dtype=np.float32)
    for b in range(4):
        s4[32 * b + n_, n_] = 1.0
    return e16, m128, md, s4


def _build_nc(t_steps=T):
    nc = bacc.Bacc(trn_type="TRN2", target_bir_lowering=False, debug=False)

    ap_xT = nc.dram_tensor("xT", [D, T * NL], BF16, kind="ExternalInput").ap()
    ap_Asc = nc.dram_tensor("Asc", [H, 512], BF16, kind="ExternalInput").ap()
    ap_Wx = nc.dram_tensor("Wx", [D, H4], BF16, kind="ExternalInput").ap()
    ap_Wh = nc.dram_tensor("Wh", [H, H4], BF16, kind="ExternalInput").ap()
    ap_Wattn = nc.dram_tensor("Wattn", [H, H4], BF16, kind="ExternalInput").ap()
    ap_bP = nc.dram_tensor("bP", [128, H4], BF16, kind="ExternalInput").ap()
    # h0T[p, 128r + 32qh + n] = h0[n, 512r + 128qh + p]; c0[32qh + n, 128r + p]
    ap_h0T = nc.dram_tensor("h0T", [128, 256], BF16, kind="ExternalInput").ap()
    ap_c0 = nc.dram_tensor("c0", [128, 256], F32, kind="ExternalInput").ap()
    # outT2[t, r, 32*qh + n, p] = h_t[n, 512r + 128qh + p]
    outT2 = nc.dram_tensor("outT2", [T, 2, 128, 128], F32, kind="ExternalOutput").ap()

    e16_np, m128_np, md_np, s4_np = _consts()
    eye_d = nc.inline_tensor(np.eye(128, dtype=np.float32), "c_eye")
    e16_d = nc.inline_tensor(e16_np, "c_e16")
    m128_d = nc.inline_tensor(m128_np, "c_m128")
    md_d = nc.inline_tensor(md_np, "c_mdiag")
    s4_d = nc.inline_tensor(s4_np, "c_s4")

    with TileContext(nc) as tc:
        with tc.tile_pool(name="pers", bufs=1) as pers:
            Wh_sb = pers.tile([128, HC * H4], BF16, tag="Wh")
            Asc_sb = pers.tile([128, HC * 512], BF16, tag="Asc")
            P_sb = pers.tile([128, 4 * H4], BF16, tag="P")
            xq = [pers.tile([128, HC * 512], BF16, tag=f"xq{b}", name=f"xq{b}")
                  for b in range(2)]
            cT = pers.tile([128, 256], F32, tag="cT")
            uThT = [pers.tile([128, 128], BF16, tag=f"uT{r}", name=f"uT{r}")
                    for r in range(2)]
            h32 = [pers.tile([128, 128], F32, tag=f"h32{r}", name=f"h32{r}")
                   for r in range(2)]
            eye = pers.tile([128, 128], F32, tag="eye")
            E16 = pers.tile([16, 128], BF16, tag="E16")
            M128 = pers.tile([128, 128], BF16, tag="M128")
            Mdiag = pers.tile([128, 512], F32, tag="Mdiag")
            S4 = pers.tile([128, 32], F32, tag="S4")
            wsq = pers.tile([32, 32], F32, tag="wsq")

            nc.sync.dma_start(eye[:], eye_d.ap()[:])
            nc.sync.dma_start(E16[:], e16_d.ap()[:])
            nc.sync.dma_start(M128[:], m128_d.ap()[:])
            nc.sync.dma_start(Mdiag[:], md_d.ap()[:])
            nc.sync.dma_start(S4[:], s4_d.ap()[:])
            nc.gpsimd.memset(wsq[:], 0.0)
            for c in range(HC):
                nc.sync.dma_start(
                    Asc_sb[:, c * 512 : (c + 1) * 512],
                    ap_Asc[128 * c : 128 * (c + 1), :],
                )

            # ---------------- phase B: P = A @ Wattn (+ b) ----------------
            with tc.tile_pool(name="phb", bufs=1) as phb, \
                 tc.tile_pool(name="psB", bufs=2, space="PSUM") as psB:
                bPsb = phb.tile([128, H4], BF16, tag="bPsb")
                nc.sync.dma_start(bPsb[:], ap_bP[:])
                # contiguous staging of the group-selected A columns so the
                # matmul stationary operand has a single free dim
                Ag = phb.tile([128, 4 * HC * 128], BF16, tag="Ag")
                for g in range(4):
                    for c in range(HC):
                        nc.vector.tensor_copy(
                            Ag[:, (g * HC + c) * 128 : (g * HC + c) * 128 + 128],
                            Asc_sb[:, c * 512 : (c + 1) * 512].rearrange(
                                "p (k n) -> p k n", k=16
                            )[:, :, 8 * g : 8 * (g + 1)],
                        )
                # recurrence-phase weight loads issued here so the transfers
                # overlap phase-B compute on other DMA rings
                for c in range(HC):
                    nc.sync.dma_start(
                        Wh_sb[:, c * H4 : (c + 1) * H4],
                        ap_Wh[128 * c : 128 * (c + 1), :],
                    )
                for c in range(HC):
                    nc.sync.dma_start(
                        xq[0][:, c * 512 : (c + 1) * 512],
                        ap_xT[128 * c : 128 * (c + 1), 0:512],
                    )
                for e in range(8):
                    Wab = phb.tile([128, HC * 512], BF16, tag=f"wab{e % 4}",
                                   name=f"wab_{e}", bufs=1)
                    for c in range(HC):
                        nc.sync.dma_start(
                            Wab[:, c * 512 : (c + 1) * 512],
                            ap_Wattn[128 * c : 128 * (c + 1), 512 * e : 512 * (e + 1)],
                        )
                    for g in range(4):
                        psp = psB.tile([128, 512], F32, tag="psp",
                                       name=f"psp_{e}_{g}")
                        for c in range(HC):
                            nc.tensor.matmul(
                                psp[:],
                                Ag[:, (g * HC + c) * 128 : (g * HC + c) * 128 + 128],
                                Wab[:, c * 512 : (c + 1) * 512],
                                start=(c == 0),
                                stop=(c == HC - 1),
                            )
                        nc.vector.tensor_add(
                            P_sb[:, g * H4 + 512 * e : g * H4 + 512 * (e + 1)],
                            psp[:],
                            bPsb[:, 512 * e : 512 * (e + 1)],
                        )

            # ---------------- h0 = c0 = mean_k(A): host-computed ----------------
            nc.sync.dma_start(uThT[0][:], ap_h0T[:, 0:128])
            nc.sync.dma_start(uThT[1][:], ap_h0T[:, 128:256])
            nc.sync.dma_start(cT[:], ap_c0[:])

            # ---------------------- recurrence ----------------------
            with tc.tile_pool(name="pers2", bufs=1) as pers2, \
                 tc.tile_pool(name="wrk", bufs=1) as wrk, \
                 tc.tile_pool(name="pstr", bufs=2, space="PSUM") as pstr, \
                 tc.tile_pool(name="pscp", bufs=1, space="PSUM") as pscp, \
                 tc.tile_pool(name="psm1", bufs=1, space="PSUM") as psm1, \
                 tc.tile_pool(name="psm3", bufs=1, space="PSUM") as psm3:
                Wx_sb = pers2.tile([128, HC * H4], BF16, tag="Wx")
                for c in range(HC):
                    nc.sync.dma_start(
                        Wx_sb[:, c * H4 : (c + 1) * H4],
                        ap_Wx[128 * c : 128 * (c + 1), :],
                    )
                # weights arrive with columns pre-permuted to (r, qh, j, p),
                # so each (c, r, qh) moving block is one contiguous 512-slice
                def wslice(W, c, r, qh):
                    base = c * H4 + (4 * r + qh) * 512
                    return W[:, base : base + 512]

                def emit_xproj(t, strips):
                    qi, tq = t // QT, t % QT
                    xv = xq[qi % 2]
                    for r in range(2):
                        for c in range(HC):
                            stat = xv[:, c * 512 + 32 * tq : c * 512 + 32 * tq + 32]
                            for qh in range(4):
                                nc.tensor.matmul(
                                    strips[r][32 * qh : 32 * (qh + 1), :],
                                    stat,
                                    wslice(Wx_sb, c, r, qh),
                                    start=(c == 0),
                                    stop=False,
                                    skip_group_check=True,
                                    tile_position=(0, 32 * qh),
                                )

                strips = [pstr.tile([128, 512], F32, tag=f"strip{r}",
                                    name=f"strip{r}_0") for r in range(2)]
                emit_xproj(0, strips)

                for t in range(t_steps):
                    # prefetch next x quarter
                    if t % QT == 0 and (t // QT) + 1 < 4 and t + QT < t_steps:
                        qn = (t // QT) + 1
                        dst = xq[qn % 2]
                        for c in range(HC):
                            nc.sync.dma_start(
                                dst[:, c * 512 : (c + 1) * 512],
                                ap_xT[128 * c : 128 * (c + 1),
                                      512 * qn : 512 * (qn + 1)],
                            )

                    # -- scores: 4 column-tiled chains, 2-deep accumulation
                    psc = pscp.tile([128, 512], F32, tag="psc", name=f"psc_{t}")
                    for c in range(HC):
                        r_c, qh_c = c // 4, c % 4
                        nc.tensor.matmul(
                            psc[32 * qh_c : 32 * (qh_c + 1), :],
                            uThT[r_c][:, 32 * qh_c : 32 * (qh_c + 1)],
                            Asc_sb[:, c * 512 : (c + 1) * 512],
                            start=(c < 4),
                            stop=(c >= 4),
                            skip_group_check=True,
                            tile_position=(0, 32 * qh_c),
                        )
                    scm = wrk.tile([128, 512], F32, tag="scm", name=f"scm_{t}")
                    nc.vector.tensor_mul(scm[:], psc[:], Mdiag[:])
                    scpart = wrk.tile([128, 16], F32, tag="scp", name=f"scp_{t}")
                    nc.vector.tensor_reduce(
                        scpart[:],
                        scm[:].rearrange("p (k n) -> p k n", k=16),
                        axis=AX.X,
                        op=ALU.add,
                    )

                    # -- gates: h @ Wh (first half; blocksum MM slots after)
                    for r in range(2):
                        for c in range(0, 4):
                            stat = uThT[c // 4][:, 32 * (c % 4) : 32 * (c % 4) + 32]
                            for qh in range(4):
                                nc.tensor.matmul(
                                    strips[r][32 * qh : 32 * (qh + 1), :],
                                    stat,
                                    wslice(Wh_sb, c, r, qh),
                                    start=False,
                                    stop=False,
                                    skip_group_check=True,
                                    tile_position=(0, 32 * qh),
                                )
                    # blocksum: scores[n, k] = sum_b scpart[32b + n, k]
                    # (scoresP + pwx share one PSUM bank)
                    psmall = psm1.tile([32, 16], F32, tag="psmall",
                                       name=f"psmall_{t}")
                    scoresP = psmall[:]
                    nc.tensor.matmul(scoresP, S4[:], scpart[:],
                                     start=True, stop=True)
                    for r in range(2):
                        for c in range(4, HC):
                            stat = uThT[c // 4][:, 32 * (c % 4) : 32 * (c % 4) + 32]
                            for qh in range(4):
                                nc.tensor.matmul(
                                    strips[r][32 * qh : 32 * (qh + 1), :],
                                    stat,
                                    wslice(Wh_sb, c, r, qh),
                                    start=False,
                                    stop=False,
                                    skip_group_check=True,
                                    tile_position=(0, 32 * qh),
                                )

                    # -- softmax (exp via sigmoid: no ACT table swap)
                    nmx = wrk.tile([32, 1], F32, tag="nmx", name=f"nmx_{t}")
                    nc.vector.tensor_reduce(
                        nmx[:], scoresP, axis=AX.X, op=ALU.max, negate=True
                    )
                    sig = wrk.tile([32, 16], F32, tag="sig", name=f"sig_{t}")
                    nc.scalar.activation(sig[:], scoresP, AF.Sigmoid,
                                         bias=nmx[:], scale=1.0)
                    om = wrk.tile([32, 16], F32, tag="om", name=f"om_{t}")
                    nc.vector.tensor_scalar(
                        om[:], sig[:], -1.0, 1.0, op0=ALU.mult, op1=ALU.add
                    )
                    rom = wrk.tile([32, 16], F32, tag="rom", name=f"rom_{t}")
                    nc.vector.reciprocal(rom[:], om[:])
                    ex = wrk.tile([32, 16], F32, tag="ex", name=f"ex_{t}")
                    nc.vector.tensor_mul(ex[:], sig[:], rom[:])
                    esum = wrk.tile([32, 1], F32, tag="esum", name=f"esum_{t}")
                    nc.vector.tensor_reduce(esum[:], ex[:], axis=AX.X, op=ALU.add)
                    rcp = wrk.tile([32, 1], F32, tag="rcp", name=f"rcp_{t}")
                    nc.vector.reciprocal(rcp[:], esum[:])
                    nc.vector.tensor_scalar_mul(wsq[:, 0:16], ex[:], rcp[:])
                    wT = wrk.tile([32, 32], F32, tag="wT", name=f"wT_{t}")
                    nc.vector.transpose(wT[:], wsq[:])
                    wTb = wrk.tile([16, 32], BF16, tag="wTb", name=f"wTb_{t}")
                    nc.vector.tensor_copy(wTb[:], wT[0:16, :])

                    # -- expand w to the (k, ng) block layout
                    pwxt = psm3.tile([128, 32], F32, tag="pwx", name=f"pwx_{t}")
                    pwx = pwxt[:]
                    nc.tensor.matmul(pwx, E16[:], wTb[:], start=True, stop=True)
                    masked = wrk.tile([128, 128], BF16, tag="masked",
                                      name=f"masked_{t}")
                    for g in range(4):
                        nc.vector.tensor_mul(
                            masked[:, 32 * g : 32 * (g + 1)],
                            pwx[:],
                            M128[:, 32 * g : 32 * (g + 1)],
                        )

                    # -- gates: attention term sum_k w_k P_k (+ b)
                    for r in range(2):
                        for g in range(4):
                            stat = masked[:, 32 * g : 32 * (g + 1)]
                            for qh in range(4):
                                nc.tensor.matmul(
                                    strips[r][32 * qh : 32 * (qh + 1), :],
                                    stat,
                                    P_sb[:, g * H4 + (4 * r + qh) * 512 :
                                         g * H4 + (4 * r + qh) * 512 + 512],
                                    start=False,
                                    stop=(g == 3),
                                    skip_group_check=True,
                                    tile_position=(0, 32 * qh),
                                )

                    # -- next step's xproj fills the PE while the tail runs
                    if t + 1 < t_steps:
                        nstrips = [pstr.tile([128, 512], F32, tag=f"strip{r}",
                                             name=f"strip{r}_{t + 1}")
                                   for r in range(2)]
                        emit_xproj(t + 1, nstrips)
                    else:
                        nstrips = None

                    # -- activations + cell update in strip space
                    pTT = psm1.tile([128, 256], F32, tag="pTT", name=f"pTT_{t}")
                    for r in range(2):
                        act = wrk.tile([128, 512], F32, tag=f"act{r}",
                                       name=f"act{r}_{t}")
                        nc.scalar.activation(act[:, 0:384], strips[r][:, 0:384],
                                             AF.Sigmoid)
                        nc.scalar.activation(act[:, 384:512], strips[r][:, 384:512],
                                             AF.Tanh)
                        cv = cT[:, 128 * r : 128 * (r + 1)]
                        ig = wrk.tile([128, 128], F32, tag=f"ig{r}",
                                      name=f"ig{r}_{t}")
                        nc.vector.tensor_mul(ig[:], act[:, 0:128], act[:, 384:512])
                        fc = wrk.tile([128, 128], F32, tag=f"fc{r}",
                                      name=f"fc{r}_{t}")
                        nc.vector.tensor_mul(fc[:], act[:, 128:256], cv)
                        nc.vector.tensor_add(cv, ig[:], fc[:])
                        tch = wrk.tile([128, 128], F32, tag=f"tch{r}",
                                       name=f"tch{r}_{t}")
                        nc.scalar.activation(tch[:], cv, AF.Tanh)
                        nc.vector.tensor_mul(h32[r][:], act[:, 256:384], tch[:])
                        nc.sync.dma_start(outT2[t, r], h32[r][:])
                        pT = pTT[:, 128 * r : 128 * (r + 1)]
                        nc.tensor.matmul(pT, h32[r][:], eye[:],
                                         is_transpose=True, start=True, stop=True)
                        nc.vector.tensor_copy(uThT[r][:], pT)

                    strips = nstrips
    nc.compile()
    return nc


def _prep_shards(inputs):
    x = np.asarray(inputs["x"], np.float32)
    A = np.asarray(inputs["A"], np.float32)
    Wx = np.asarray(inputs["Wx"], np.float32)
    Wh = np.asarray(inputs["Wh"], np.float32)
    Wattn = np.asarray(inputs["Wattn"], np.float32)
    b = np.asarray(inputs["b"], np.float32)

    # permute gate columns (j, r, qh, p) -> (r, qh, j, p) so device moving
    # blocks are contiguous
    def _perm(W):
        return np.ascontiguousarray(
            W.reshape(-1, 4, 2, 4, 128).transpose(0, 2, 3, 1, 4).reshape(-1, H4)
        )

    Wx_bf = _perm(Wx).astype(BF)
    Wh_bf = _perm(Wh).astype(BF)
    Wa_bf = _perm(Wattn).astype(BF)
    bp = _perm(b.reshape(1, H4)).reshape(H4)
    bP = np.ascontiguousarray(np.tile(bp.astype(BF)[None, :], (128, 1)))

    in_maps = []
    for i in range(NCORES):
        ns = slice(NL * i, NL * (i + 1))
        xT = x[ns].transpose(2, 1, 0).reshape(D, T * NL)
        Asc = A[ns].reshape(NL, H, 16).transpose(1, 2, 0).reshape(H, 512)
        h0 = A[ns].reshape(NL, H, 16).mean(axis=2)          # [32, 1024]
        h05 = h0.reshape(NL, 2, 4, 128)                     # [n, r, qh, p]
        h0T = h05.transpose(3, 1, 2, 0).reshape(128, 256)   # [p, (r qh n)]
        c0 = h05.transpose(2, 0, 1, 3).reshape(128, 256)    # [(qh n), (r p)]
        in_maps.append(
            {
                "xT": np.ascontiguousarray(xT.astype(BF)),
                "Asc": np.ascontiguousarray(Asc.astype(BF)),
                "Wx": Wx_bf,
                "Wh": Wh_bf,
                "Wattn": Wa_bf,
                "bP": bP,
                "h0T": np.ascontiguousarray(h0T.astype(BF)),
                "c0": np.ascontiguousarray(c0.astype(np.float32)),
            }
        )
    return in_maps


def _get_nc():
    global _built
    if _built is None:
        _built = _build_nc()
    return _built


def _run(inputs, **kwargs):
    nc = _get_nc()
    in_maps = _prep_shards(inputs)
    res = bass_utils.run_bass_kernel_spmd(
        nc, in_maps, core_ids=list(range(NCORES)), **kwargs
    )
    out = np.empty((N, T, H), np.float32)
    for i in range(NCORES):
        o = res.results[i]["outT2"]  # [T, 2, 128, 128] f32
        out[NL * i : NL * (i + 1)] = (
            o.reshape(T, 2, 4, 32, 128).transpose(3, 0, 1, 2, 4).reshape(NL, T, H)
        )
    return out, res


def kernel(**inputs):
    out, _ = _run(inputs)
    return out
